# revision 46
# baseline (speedup 1.0000x reference)
"""Two-branch attention kernel for Trainium2 (8 NeuronCores, batch-parallel).

out1 = proj(softmax(q k^T / 8) v),  out2 = proj(softmax(q k2^T / 8) v2)
with q,k,v from x and k2,v2 from x2 (q shared across branches).

Sharding: batch dim (8) -> one batch element per core. No collectives.

Design (vs f32r baseline at 544us; this kernel ~355us at full clock):
  * all matmul operands bf16: halves DMA + SBUF, no DRAM spill of k2/v2
    (everything resident), faster LDWEIGHTS. rel err ~6e-3 (budget 2e-2).
  * S psum split into [P,2,512] half-tiles double-buffered so exp (ACT)
    overlaps the next S matmul instead of serializing the PE; keeping the
    PE gap-free also keeps it at the 2.4GHz p-state (vs 1.2GHz).
  * softmax normalization decoupled from the AV accumulators: unnormalized
    o + row-sum rows are evicted immediately (frees PSUM), row sums are
    gathered via DRAM-bounce DMAs into [8,128]-reshaped batch tiles so one
    batched DVE reciprocal covers 6 heads in ~0.9us (reciprocal costs
    free-size cycles: 6.5us for [n,1024], regardless of n), then 1/r is
    partition-broadcast and applied in-place on DVE.
  * emission-order software pipelining: QKV for x2 (k2T/v2) interleaved
    into branch-1 attention; branch-1 proj into branch-2 attention; S/exp
    for the next (branch,g) unit interleaved into the current unit's AV;
    all PSUM evictions on DVE so ACT does nothing but exp in steady state;
    4 branch-0 projs reserved for the tail (from the idle S psum pool) to
    cover the last normalize chain.
"""
import sys
for _p in ('/opt/trn_rl_repo',):
    if _p not in sys.path:
        sys.path.insert(0, _p)

import numpy as np

MODE = 'bf16-pipelined'

B, N, D, H, HD = 8, 1024, 768, 12, 64
SCALE = HD ** -0.5
NDT = D // 128       # 6 dim tiles
NQT = N // 128       # 8 token tiles
P = 128
AUG = HD + 1         # 65: head dim + ones column for row sums
NU = 12              # (branch, g) attention units


# ----------------------------------------------------------------------------
# workaround: walrus rejects >2 sem waits on one instruction; TileContext's
# tail drain carries one wait per active logical proc. Split them across
# single-wait SP nops and emit a bare drain.
def _install_tilefix():
    import bass_rust
    import concourse.tile as tile

    def _drain_and_barrier_split(self, tick_clock, wait_clock):
        gc = tick_clock.global_clock
        ticks = [gc[i] for i in range(27)]
        for i, t in enumerate(ticks):
            if t > 0:
                vc = bass_rust.VectorClock(
                    [t if j == i else 0 for j in range(len(ticks))])
                nop = self.nc.sync.nop()
                wait_clock.add_sem_waits(
                    nop.ins, bass_rust.ScopedClock({None: vc}))
        self.nc.sync.drain()
        self.nc.all_engine_barrier()
        assert self.sems is not None
        popped = self.nc._tile_sem_poison_stack.pop()
        assert popped is self._sem_poison
        self.nc.clear_and_free_semaphores(list(self.sems.allocated().values()))
        self.nc.all_engine_barrier()

    tile.TileContext._drain_and_barrier = _drain_and_barrier_split


def _split_multiwaits(nc, max_waits=1):
    """walrus codegen rejects instructions carrying more than `max_waits`
    sync waits; hoist the extras onto same-engine nops placed just before."""
    import bass_rust
    import concourse.mybir as mybir
    cnt = 0
    for bb in nc.main_func.blocks:
        insts = bb.instructions
        i = 0
        while i < len(insts):
            ins = insts[i]
            si = getattr(ins, 'sync_info', None)
            if si is not None and si.on_wait and len(si.on_wait) > max_waits:
                waits = list(si.on_wait)
                extras, keep = waits[:-max_waits], waits[-max_waits:]
                for w in extras:
                    nop = mybir.InstNoOp(name=f"I-swx{cnt}", ins=[], outs=[])
                    cnt += 1
                    nop.engine = ins.engine
                    nop.sync_info = bass_rust.SyncInfo(on_wait=[w],
                                                       on_update=[])
                    insts.insert(i, nop)
                    i += 1
                ins.sync_info = bass_rust.SyncInfo(
                    on_wait=keep, on_update=list(si.on_update))
            i += 1
    return cnt


_built = None


def _build():
    """Build the SPMD bass program once. Returns (nc, n_split_waits)."""
    global _built
    if _built is not None:
        return _built
    _install_tilefix()
    from contextlib import ExitStack
    import concourse.bass as bass
    import concourse.tile as tile
    from concourse import mybir

    dt = mybir.dt
    mdt = dt.bfloat16          # matmul operand dtype everywhere

    nc = bass.Bass("TRN2", target_bir_lowering=False, debug=False,
                   num_devices=8)

    # DRAM I/O (per core)
    xt_d = nc.dram_tensor("xt", [D, N], mdt, kind="ExternalInput")
    x2t_d = nc.dram_tensor("x2t", [D, N], mdt, kind="ExternalInput")
    wqk_d = nc.dram_tensor("wqk", [D, 2 * D], mdt, kind="ExternalInput")
    wv_d = nc.dram_tensor("wv", [D, D], mdt, kind="ExternalInput")
    wp_d = nc.dram_tensor("wp", [D, D], mdt, kind="ExternalInput")
    bias_d = nc.dram_tensor("bias", [P, D], dt.float32, kind="ExternalInput")
    ones_d = nc.dram_tensor("ones", [P, H, 1], mdt, kind="ExternalInput")
    out_d = nc.dram_tensor("out", [2, N, D], dt.float32,
                           kind="ExternalOutput")

    with tile.TileContext(nc) as tc, ExitStack() as top:
        # PSUM: pp (S half-tiles + QKV groups, 2KB ea) 4 banks,
        #       pp_o (AV accumulators + proj) 4 banks.
        pp = top.enter_context(tc.tile_pool(name="ps", bufs=2, space="PSUM"))
        pp_o = top.enter_context(tc.tile_pool(name="ps_o", bufs=2,
                                              space="PSUM"))
        dram_rb = top.enter_context(tc.tile_pool(name="dram_rb", bufs=2,
                                                 space="DRAM"))
        persist = top.enter_context(tc.tile_pool(name="persist", bufs=1))
        pool_pt = top.enter_context(tc.tile_pool(name="pt", bufs=5))
        pool_sm = top.enter_context(tc.tile_pool(name="sm", bufs=2))
        pool_rv = top.enter_context(tc.tile_pool(name="rv", bufs=1))
        pool_res = top.enter_context(tc.tile_pool(name="res", bufs=2))

        # persistent SBUF tiles (bf16): ~104KB/partition
        qT = persist.tile([P, NDT, N], mdt, tag="qT")
        kT1 = persist.tile([P, NDT, N], mdt, tag="kT1")
        kT2 = persist.tile([P, NDT, N], mdt, tag="kT2")
        vaug1 = persist.tile([P, NQT, H * AUG], mdt, tag="va1")
        vaug2 = persist.tile([P, NQT, H * AUG], mdt, tag="va2")
        wp_t = persist.tile([P, NDT, D], mdt, tag="wp")
        bias_t = persist.tile([P, D], dt.float32, tag="bias")
        ot = [persist.tile([P, NDT, N], mdt, tag=f"ot{b}", name=f"ot{b}")
              for b in (0, 1)]
        # r rows, gathered via SBUF-to-SBUF DMA (engines cannot write at
        # arbitrary partition bases; DMA can) and reshaped [row,1024] ->
        # [8 partitions,128] so the slow reciprocal runs partition-parallel.
        # br0 batches complete at units 2/5; br1 at 8/10/11 (small last
        # batch keeps the tail chain short).
        BATCHES = {0: [(0, 1, 2), (3, 4, 5)],
                   1: [(0, 1, 2), (3,), (4,), (5,)]}
        G2B = {br: {g: (bi, list(gs).index(g))
                    for bi, gs in enumerate(BATCHES[br]) for g in gs}
               for br in (0, 1)}
        # [8 partitions, 128] blocks per r-vector: the slow DVE reciprocal
        # costs free-size cycles, so folding tokens onto partitions makes
        # the batched reciprocal ~8x cheaper. Gather goes through DRAM
        # (SBUF->SBUF partition-reshape DMAs fail to load).
        rall = {(br, bi): persist.tile([16 * len(gs), P], mdt,
                                       tag=f"rall{br}{bi}",
                                       name=f"rall{br}{bi}")
                for br in (0, 1) for bi, gs in enumerate(BATCHES[br])}

        # phase-A inputs (innermost pool; closed once QKV emission is done)
        pha = top.enter_context(tc.tile_pool(name="pha", bufs=1))
        xt_t = pha.tile([P, NDT, N], mdt, tag="xt")
        x2t_t = pha.tile([P, NDT, N], mdt, tag="x2t")
        wqk_t = pha.tile([P, NDT, 2 * D], mdt, tag="wqk")
        wv_t = pha.tile([P, NDT, D], mdt, tag="wv")

        # input DMAs, priority order; q-columns chunked per output tile so
        # the first matmul group starts after ~1/12 of the weights arrive
        nc.sync.dma_start(
            out=wqk_t[:, :, 0:P],
            in_=wqk_d[:, 0:P].rearrange("(i p) d -> p i d", p=P))
        for i in range(NDT):
            nc.sync.dma_start(out=xt_t[:, i, :],
                              in_=xt_d[i * P:(i + 1) * P, :])
        for o in range(1, NDT):
            nc.sync.dma_start(
                out=wqk_t[:, :, o * P:(o + 1) * P],
                in_=wqk_d[:, o * P:(o + 1) * P].rearrange(
                    "(i p) d -> p i d", p=P))
        nc.sync.dma_start(
            out=wqk_t[:, :, D:2 * D],
            in_=wqk_d[:, D:2 * D].rearrange("(i p) d -> p i d", p=P))
        nc.sync.dma_start(out=wv_t,
                          in_=wv_d[:].rearrange("(i p) d -> p i d", p=P))
        nc.sync.dma_start(out=x2t_t,
                          in_=x2t_d[:].rearrange("(i p) n -> p i n", p=P))
        nc.sync.dma_start(
            out=wp_t, in_=wp_d[:].rearrange("(g p) d -> p g d", p=P))
        nc.sync.dma_start(out=bias_t, in_=bias_d[:])
        for va in (vaug1, vaug2):
            for t in range(NQT):
                nc.sync.dma_start(
                    out=va[:, t, :].rearrange("p (h e) -> p h e",
                                              e=AUG)[:, :, HD:AUG],
                    in_=ones_d[:])

        # ---------------- QKV emit units --------------------------------
        def qkT_group(src_x, wcol0, dst, o):
            """one [128,1024] output tile of q^T/k^T via W-stationary."""
            psf = pp.tile([P, 2, 512], dt.float32, tag="S")
            ps = psf.rearrange("p a n -> p (a n)")
            for i in range(NDT):
                wsl = wqk_t[:, i, wcol0 + o * P: wcol0 + (o + 1) * P]
                for c in range(2):
                    nc.tensor.matmul(
                        ps[:, c * 512:(c + 1) * 512], wsl,
                        src_x[:, i, c * 512:(c + 1) * 512],
                        start=(i == 0), stop=(i == NDT - 1))
            nc.vector.tensor_copy(dst[:, o, :], ps[:])

        def v_group(src_x, vaug_t, t):
            """one [128tok, 768] v tile via x-stationary into vaug."""
            psf = pp.tile([P, 2, 512], dt.float32, tag="S")
            ps = psf.rearrange("p a n -> p (a n)")
            for i in range(NDT):
                xsl = src_x[:, i, t * P:(t + 1) * P]
                for c0, cn in ((0, 512), (512, 256)):
                    nc.tensor.matmul(
                        ps[:, c0:c0 + cn], xsl, wv_t[:, i, c0:c0 + cn],
                        start=(i == 0), stop=(i == NDT - 1))
            src = ps[:, 0:D].rearrange("p (h e) -> p h e", e=HD)
            dst = vaug_t[:, t, :].rearrange("p (h e) -> p h e",
                                            e=AUG)[:, :, 0:HD]
            nc.vector.tensor_copy(dst, src)

        # ---------------- attention units -------------------------------
        units = [(0, g) for g in range(NDT)] + [(1, g) for g in range(NDT)]
        kTs, vas = (kT1, kT2), (vaug1, vaug2)
        pt_tiles = {}   # (u, kjp) -> tile [P, 2, 2, N]

        def part1(u, kjp):
            """S + exp for kj pair kjp of unit u -> pt tile (bf16)."""
            br, g = units[u]
            kT_t = kTs[br]
            pt = pool_pt.tile([P, 2, 2, N], mdt, tag="pt")
            pt_tiles[(u, kjp)] = pt
            for kjl in range(2):
                kj = 2 * kjp + kjl
                for c in range(2):
                    sc = pp.tile([P, 2, 512], dt.float32, tag="S")
                    for hh in range(2):
                        r0 = hh * HD
                        nc.tensor.matmul(
                            sc[:, hh, :],
                            kT_t[r0:r0 + HD, g, kj * P:(kj + 1) * P],
                            qT[r0:r0 + HD, g, c * 512:(c + 1) * 512],
                            start=True, stop=True, skip_group_check=True)
                    nc.scalar.activation(
                        pt[:, :, kjl, c * 512:(c + 1) * 512], sc[:],
                        mybir.ActivationFunctionType.Exp, scale=SCALE)

        def emit_av(u, po, kjp):
            br, g = units[u]
            va = vas[br]
            pt = pt_tiles[(u, kjp)]
            for kjl in range(2):
                kj = 2 * kjp + kjl
                for hh in range(2):
                    h = 2 * g + hh
                    for c in range(2):
                        nc.tensor.matmul(
                            po[hh][0:AUG, c * 512:(c + 1) * 512],
                            va[:, kj, h * AUG:(h + 1) * AUG],
                            pt[:, hh, kjl, c * 512:(c + 1) * 512],
                            start=(kj == 0), stop=(kj == NQT - 1),
                            skip_group_check=True)

        def unit_copies(u, po):
            """evict AV result (unnormalized) + its row-sums; frees po.
            The last unit evicts on ACT (idle there) to shorten the tail."""
            br, g = units[u]
            bi, j = G2B[br][g]
            cp = nc.scalar.copy if u == NU - 1 else nc.vector.tensor_copy
            for hh in range(2):
                cp(ot[br][hh * HD:(hh + 1) * HD, g, :], po[hh][0:HD, :])
                rt = pool_sm.tile([1, N], mdt, tag="rt")
                cp(rt[:], po[hh][HD:HD + 1, :])
                rw = dram_rb.tile([8, P], mdt, tag="rw")
                nc.sync.dma_start(out=rw[:], in_=rt[:])
                row = 16 * j + 8 * hh
                nc.sync.dma_start(out=rall[(br, bi)][row:row + 8, :],
                                  in_=rw[:])

        def norm_batch(br, bi):
            """batched 1/r (partition-parallel) + broadcast + in-place scale."""
            gs = BATCHES[br][bi]
            rinv = pool_rv.tile([16 * len(gs), P], dt.float32, tag="rinv",
                                padded_shape=[48, P])
            nc.vector.reciprocal(rinv[:], rall[(br, bi)][:])
            rd = dram_rb.tile([2 * len(gs), N], dt.float32, tag="rd",
                              padded_shape=[6, N])
            nc.sync.dma_start(out=rd[:], in_=rinv[:])
            for jj, g in enumerate(gs):
                rb = pool_sm.tile([P, N], dt.float32, tag="rb")
                # 32-partition slices: partition_broadcast is DMA-descriptor
                # bound (~1/partition), so split across 4 parallel queues
                for q in range(4):
                    nc.sync.dma_start(
                        out=rb[q * 32:(q + 1) * 32, :],
                        in_=rd[2 * jj + q // 2, :].partition_broadcast(32))
                sl = ot[br][:, g, :]
                nc.vector.tensor_tensor(sl, sl, rb[:],
                                        mybir.AluOpType.mult)

        def proj_qi(br, qi, pool=None):
            if pool is None:
                psf = pp_o.tile([P, N], dt.float32, tag="O")
                ps = psf[:, 0:D]
            else:
                # tail projs run from the (idle) S pool so they don't wait
                # on the last unit's AV accumulators being evicted
                psf = pool.tile([P, 2, 512], dt.float32, tag="S")
                ps = psf.rearrange("p a n -> p (a n)")[:, 0:D]
            for g in range(NDT):
                osl = ot[br][:, g, qi * P:(qi + 1) * P]
                for c0, cn in ((0, 512), (512, 256)):
                    nc.tensor.matmul(
                        ps[:, c0:c0 + cn], osl, wp_t[:, g, c0:c0 + cn],
                        start=(g == 0), stop=(g == NDT - 1),
                        skip_group_check=True)
            res = pool_res.tile([P, D], dt.float32, tag="res")
            nc.vector.tensor_add(res[:], ps[:], bias_t[:])
            nc.sync.dma_start(out=out_d[br, qi * P:(qi + 1) * P, :],
                              in_=res[:])

        # ---------------- emission schedule -----------------------------
        # QKV-x: q^T, k^T
        for o in range(NDT):
            qkT_group(xt_t, 0, qT, o)
        for o in range(NDT):
            qkT_group(xt_t, D, kT1, o)
        # v interleaved with S/exp of unit 0 (needs only qT/kT1)
        for t in range(NQT):
            v_group(xt_t, vaug1, t)
            if t % 2 == 1:
                part1(0, t // 2)

        # mid-attention fillers: x2 QKV during branch-1, proj(br0) during
        # branch-2.  Safe points: QKV fillers mid-unit (depend only on the
        # past); proj fillers only at unit end (they wait on normalize).
        qkv_fill = ([(lambda o=o: qkT_group(x2t_t, D, kT2, o))
                     for o in range(NDT)]
                    + [(lambda t=t: v_group(x2t_t, vaug2, t))
                       for t in range(NQT)])
        proj_fill = []

        for u in range(NU):
            po = [pp_o.tile([P, N], dt.float32, tag="O",
                            name=f"po{u}_{hh}") for hh in range(2)]
            for kjp in range(4):
                emit_av(u, po, kjp)
                if u + 1 < NU:
                    part1(u + 1, kjp)
                # 2 fillers/unit over units 0-6 (vs 3/unit over 0-4):
                # pushes the last x2-QKV groups into the ACT-bound branch-2
                # region where the PE otherwise idles on exp waits. v2
                # tiles t6/t7 land at unit 6 kjp 1/2, still ahead of the
                # AV(kjp=3) that reads them.
                if kjp in (1, 2) and qkv_fill:
                    qkv_fill.pop(0)()
            unit_copies(u, po)
            br, g = units[u]
            for bi, gs in enumerate(BATCHES[br]):
                if g == gs[-1]:
                    norm_batch(br, bi)
            if u == 5:
                proj_fill = [(lambda qi=qi: proj_qi(0, qi))
                             for qi in range(NQT)]
            for _ in range({7: 1, 8: 1, 9: 1, 10: 1}.get(u, 0)):
                if proj_fill:
                    proj_fill.pop(0)()
        # tail: leftover br0 projs fill the last normalize window (from the
        # S psum pool, so they don't wait on the last AV eviction); then the
        # first two br1 projs run as g0-g4 partials during the g5 normalize
        # chain, finished with the g5 contribution once its scale lands.
        for qi in range(NQT - len(proj_fill), NQT):
            proj_qi(0, qi, pool=pp)

        def proj_partial(qi):
            psf = pp.tile([P, 2, 512], dt.float32, tag="S")
            ps = psf.rearrange("p a n -> p (a n)")[:, 0:D]
            for g in range(NDT - 1):
                osl = ot[1][:, g, qi * P:(qi + 1) * P]
                for c0, cn in ((0, 512), (512, 256)):
                    nc.tensor.matmul(
                        ps[:, c0:c0 + cn], osl, wp_t[:, g, c0:c0 + cn],
                        start=(g == 0), stop=False, skip_group_check=True)
            return ps

        def proj_finish(qi, ps):
            g = NDT - 1
            osl = ot[1][:, g, qi * P:(qi + 1) * P]
            for c0, cn in ((0, 512), (512, 256)):
                nc.tensor.matmul(
                    ps[:, c0:c0 + cn], osl, wp_t[:, g, c0:c0 + cn],
                    start=False, stop=True, skip_group_check=True)
            res = pool_res.tile([P, D], dt.float32, tag="res")
            nc.vector.tensor_add(res[:], ps[:], bias_t[:])
            nc.sync.dma_start(out=out_d[1, qi * P:(qi + 1) * P, :],
                              in_=res[:])

        parts = [proj_partial(qi) for qi in (0, 1)]
        for qi in (0, 1):
            proj_finish(qi, parts[qi])
        for qi in range(2, NQT):
            proj_qi(1, qi)

    n = _split_multiwaits(nc)
    _built = (nc, n)
    return _built


def _host_prep(x, x2, qkv_w, proj_w, proj_b):
    """-> list of 8 per-core input maps (bf16 operands, f32 bias)."""
    import ml_dtypes
    bf = lambda a: np.ascontiguousarray(np.asarray(a),
                                        ).astype(ml_dtypes.bfloat16)

    xt = np.transpose(np.asarray(x), (0, 2, 1))
    x2t = np.transpose(np.asarray(x2), (0, 2, 1))
    wqk = bf(np.asarray(qkv_w)[:2 * D].T)       # [768, 1536]
    wv = bf(np.asarray(qkv_w)[2 * D:].T)        # [768, 768]
    wp = bf(np.asarray(proj_w).T)               # [768, 768]
    bias = np.broadcast_to(np.asarray(proj_b, dtype=np.float32),
                           (P, D)).copy()
    ones = np.ones((P, H, 1), dtype=ml_dtypes.bfloat16)
    maps = []
    for c in range(B):
        maps.append({
            "xt": bf(xt[c]), "x2t": bf(x2t[c]),
            "wqk": wqk, "wv": wv, "wp": wp, "bias": bias,
            "ones": ones,
        })
    return maps


def kernel(x, x2, qkv_w, proj_w, proj_b, trace=False, tmpdir=None):
    nc, _ = _build()
    from concourse.bass_utils import run_bass_kernel_spmd
    in_maps = _host_prep(x, x2, qkv_w, proj_w, proj_b)
    res = run_bass_kernel_spmd(nc, in_maps, list(range(B)), trace=trace,
                               tmpdir=tmpdir)
    kernel.last_exec_time_ns = res.exec_time_ns
    out = np.stack([res.results[c]["out"] for c in range(B)])  # [B,2,N,D]
    out1 = np.ascontiguousarray(out[:, 0])
    out2 = np.ascontiguousarray(out[:, 1])
    return (out1, out2)


kernel.last_exec_time_ns = None


# revision 47
# speedup vs baseline: 1.0031x; 1.0031x over previous
"""Two-branch attention kernel for Trainium2 (8 NeuronCores, batch-parallel).

out1 = proj(softmax(q k^T / 8) v),  out2 = proj(softmax(q k2^T / 8) v2)
with q,k,v from x and k2,v2 from x2 (q shared across branches).

Sharding: batch dim (8) -> one batch element per core. No collectives.

Design (vs f32r baseline at 544us; this kernel ~355us at full clock):
  * all matmul operands bf16: halves DMA + SBUF, no DRAM spill of k2/v2
    (everything resident), faster LDWEIGHTS. rel err ~6e-3 (budget 2e-2).
  * S psum split into [P,2,512] half-tiles double-buffered so exp (ACT)
    overlaps the next S matmul instead of serializing the PE; keeping the
    PE gap-free also keeps it at the 2.4GHz p-state (vs 1.2GHz).
  * softmax normalization decoupled from the AV accumulators: unnormalized
    o + row-sum rows are evicted immediately (frees PSUM), row sums are
    gathered via DRAM-bounce DMAs into [8,128]-reshaped batch tiles so one
    batched DVE reciprocal covers 6 heads in ~0.9us (reciprocal costs
    free-size cycles: 6.5us for [n,1024], regardless of n), then 1/r is
    partition-broadcast and applied in-place on DVE.
  * emission-order software pipelining: QKV for x2 (k2T/v2) interleaved
    into branch-1 attention; branch-1 proj into branch-2 attention; S/exp
    for the next (branch,g) unit interleaved into the current unit's AV;
    all PSUM evictions on DVE so ACT does nothing but exp in steady state;
    4 branch-0 projs reserved for the tail (from the idle S psum pool) to
    cover the last normalize chain.
"""
import sys
for _p in ('/opt/trn_rl_repo',):
    if _p not in sys.path:
        sys.path.insert(0, _p)

import numpy as np

MODE = 'bf16-pipelined'

B, N, D, H, HD = 8, 1024, 768, 12, 64
SCALE = HD ** -0.5
NDT = D // 128       # 6 dim tiles
NQT = N // 128       # 8 token tiles
P = 128
AUG = HD + 1         # 65: head dim + ones column for row sums
NU = 12              # (branch, g) attention units


# ----------------------------------------------------------------------------
# workaround: walrus rejects >2 sem waits on one instruction; TileContext's
# tail drain carries one wait per active logical proc. Split them across
# single-wait SP nops and emit a bare drain.
def _install_tilefix():
    import bass_rust
    import concourse.tile as tile

    def _drain_and_barrier_split(self, tick_clock, wait_clock):
        gc = tick_clock.global_clock
        ticks = [gc[i] for i in range(27)]
        for i, t in enumerate(ticks):
            if t > 0:
                vc = bass_rust.VectorClock(
                    [t if j == i else 0 for j in range(len(ticks))])
                nop = self.nc.sync.nop()
                wait_clock.add_sem_waits(
                    nop.ins, bass_rust.ScopedClock({None: vc}))
        self.nc.sync.drain()
        self.nc.all_engine_barrier()
        assert self.sems is not None
        popped = self.nc._tile_sem_poison_stack.pop()
        assert popped is self._sem_poison
        self.nc.clear_and_free_semaphores(list(self.sems.allocated().values()))
        self.nc.all_engine_barrier()

    tile.TileContext._drain_and_barrier = _drain_and_barrier_split


def _split_multiwaits(nc, max_waits=1):
    """walrus codegen rejects instructions carrying more than `max_waits`
    sync waits; hoist the extras onto same-engine nops placed just before."""
    import bass_rust
    import concourse.mybir as mybir
    cnt = 0
    for bb in nc.main_func.blocks:
        insts = bb.instructions
        i = 0
        while i < len(insts):
            ins = insts[i]
            si = getattr(ins, 'sync_info', None)
            if si is not None and si.on_wait and len(si.on_wait) > max_waits:
                waits = list(si.on_wait)
                extras, keep = waits[:-max_waits], waits[-max_waits:]
                for w in extras:
                    nop = mybir.InstNoOp(name=f"I-swx{cnt}", ins=[], outs=[])
                    cnt += 1
                    nop.engine = ins.engine
                    nop.sync_info = bass_rust.SyncInfo(on_wait=[w],
                                                       on_update=[])
                    insts.insert(i, nop)
                    i += 1
                ins.sync_info = bass_rust.SyncInfo(
                    on_wait=keep, on_update=list(si.on_update))
            i += 1
    return cnt


_built = None


def _build():
    """Build the SPMD bass program once. Returns (nc, n_split_waits)."""
    global _built
    if _built is not None:
        return _built
    _install_tilefix()
    from contextlib import ExitStack
    import concourse.bass as bass
    import concourse.tile as tile
    from concourse import mybir

    dt = mybir.dt
    mdt = dt.bfloat16          # matmul operand dtype everywhere

    nc = bass.Bass("TRN2", target_bir_lowering=False, debug=False,
                   num_devices=8)

    # DRAM I/O (per core)
    xt_d = nc.dram_tensor("xt", [D, N], mdt, kind="ExternalInput")
    x2t_d = nc.dram_tensor("x2t", [D, N], mdt, kind="ExternalInput")
    wqk_d = nc.dram_tensor("wqk", [D, 2 * D], mdt, kind="ExternalInput")
    wv_d = nc.dram_tensor("wv", [D, D], mdt, kind="ExternalInput")
    wp_d = nc.dram_tensor("wp", [D, D], mdt, kind="ExternalInput")
    bias_d = nc.dram_tensor("bias", [P, D], dt.float32, kind="ExternalInput")
    ones_d = nc.dram_tensor("ones", [P, H, 1], mdt, kind="ExternalInput")
    out_d = nc.dram_tensor("out", [2, N, D], dt.float32,
                           kind="ExternalOutput")

    with tile.TileContext(nc) as tc, ExitStack() as top:
        # PSUM: pp (S half-tiles + QKV groups, 2KB ea) 4 banks,
        #       pp_o (AV accumulators + proj) 4 banks.
        pp = top.enter_context(tc.tile_pool(name="ps", bufs=2, space="PSUM"))
        pp_o = top.enter_context(tc.tile_pool(name="ps_o", bufs=2,
                                              space="PSUM"))
        dram_rb = top.enter_context(tc.tile_pool(name="dram_rb", bufs=2,
                                                 space="DRAM"))
        persist = top.enter_context(tc.tile_pool(name="persist", bufs=1))
        pool_pt = top.enter_context(tc.tile_pool(name="pt", bufs=5))
        pool_sm = top.enter_context(tc.tile_pool(name="sm", bufs=2))
        pool_rv = top.enter_context(tc.tile_pool(name="rv", bufs=1))
        pool_res = top.enter_context(tc.tile_pool(name="res", bufs=2))

        # persistent SBUF tiles (bf16): ~104KB/partition
        qT = persist.tile([P, NDT, N], mdt, tag="qT")
        kT1 = persist.tile([P, NDT, N], mdt, tag="kT1")
        kT2 = persist.tile([P, NDT, N], mdt, tag="kT2")
        vaug1 = persist.tile([P, NQT, H * AUG], mdt, tag="va1")
        vaug2 = persist.tile([P, NQT, H * AUG], mdt, tag="va2")
        wp_t = persist.tile([P, NDT, D], mdt, tag="wp")
        bias_t = persist.tile([P, D], dt.float32, tag="bias")
        ot = [persist.tile([P, NDT, N], mdt, tag=f"ot{b}", name=f"ot{b}")
              for b in (0, 1)]
        # r rows, gathered via SBUF-to-SBUF DMA (engines cannot write at
        # arbitrary partition bases; DMA can) and reshaped [row,1024] ->
        # [8 partitions,128] so the slow reciprocal runs partition-parallel.
        # br0 batches complete at units 2/5; br1 at 8/10/11 (small last
        # batch keeps the tail chain short).
        BATCHES = {0: [(0, 1, 2), (3, 4, 5)],
                   1: [(0, 1, 2), (3,), (4,), (5,)]}
        G2B = {br: {g: (bi, list(gs).index(g))
                    for bi, gs in enumerate(BATCHES[br]) for g in gs}
               for br in (0, 1)}
        # [8 partitions, 128] blocks per r-vector: the slow DVE reciprocal
        # costs free-size cycles, so folding tokens onto partitions makes
        # the batched reciprocal ~8x cheaper. Gather goes through DRAM
        # (SBUF->SBUF partition-reshape DMAs fail to load).
        rall = {(br, bi): persist.tile([16 * len(gs), P], mdt,
                                       tag=f"rall{br}{bi}",
                                       name=f"rall{br}{bi}")
                for br in (0, 1) for bi, gs in enumerate(BATCHES[br])}

        # phase-A inputs (innermost pool; closed once QKV emission is done)
        pha = top.enter_context(tc.tile_pool(name="pha", bufs=1))
        xt_t = pha.tile([P, NDT, N], mdt, tag="xt")
        x2t_t = pha.tile([P, NDT, N], mdt, tag="x2t")
        wqk_t = pha.tile([P, NDT, 2 * D], mdt, tag="wqk")
        wv_t = pha.tile([P, NDT, D], mdt, tag="wv")

        # input DMAs, priority order; q-columns chunked per output tile so
        # the first matmul group starts after ~1/12 of the weights arrive
        nc.sync.dma_start(
            out=wqk_t[:, :, 0:P],
            in_=wqk_d[:, 0:P].rearrange("(i p) d -> p i d", p=P))
        for i in range(NDT):
            nc.sync.dma_start(out=xt_t[:, i, :],
                              in_=xt_d[i * P:(i + 1) * P, :])
        for o in range(1, NDT):
            nc.sync.dma_start(
                out=wqk_t[:, :, o * P:(o + 1) * P],
                in_=wqk_d[:, o * P:(o + 1) * P].rearrange(
                    "(i p) d -> p i d", p=P))
        nc.sync.dma_start(
            out=wqk_t[:, :, D:2 * D],
            in_=wqk_d[:, D:2 * D].rearrange("(i p) d -> p i d", p=P))
        nc.sync.dma_start(out=wv_t,
                          in_=wv_d[:].rearrange("(i p) d -> p i d", p=P))
        nc.sync.dma_start(out=x2t_t,
                          in_=x2t_d[:].rearrange("(i p) n -> p i n", p=P))
        nc.sync.dma_start(
            out=wp_t, in_=wp_d[:].rearrange("(g p) d -> p g d", p=P))
        nc.sync.dma_start(out=bias_t, in_=bias_d[:])
        for va in (vaug1, vaug2):
            for t in range(NQT):
                nc.sync.dma_start(
                    out=va[:, t, :].rearrange("p (h e) -> p h e",
                                              e=AUG)[:, :, HD:AUG],
                    in_=ones_d[:])

        # ---------------- QKV emit units --------------------------------
        def qkT_group(src_x, wcol0, dst, o):
            """one [128,1024] output tile of q^T/k^T via W-stationary."""
            psf = pp.tile([P, 2, 512], dt.float32, tag="S")
            ps = psf.rearrange("p a n -> p (a n)")
            for i in range(NDT):
                wsl = wqk_t[:, i, wcol0 + o * P: wcol0 + (o + 1) * P]
                for c in range(2):
                    nc.tensor.matmul(
                        ps[:, c * 512:(c + 1) * 512], wsl,
                        src_x[:, i, c * 512:(c + 1) * 512],
                        start=(i == 0), stop=(i == NDT - 1))
            nc.vector.tensor_copy(dst[:, o, :], ps[:])

        def v_group(src_x, vaug_t, t):
            """one [128tok, 768] v tile via x-stationary into vaug."""
            psf = pp.tile([P, 2, 512], dt.float32, tag="S")
            ps = psf.rearrange("p a n -> p (a n)")
            for i in range(NDT):
                xsl = src_x[:, i, t * P:(t + 1) * P]
                for c0, cn in ((0, 512), (512, 256)):
                    nc.tensor.matmul(
                        ps[:, c0:c0 + cn], xsl, wv_t[:, i, c0:c0 + cn],
                        start=(i == 0), stop=(i == NDT - 1))
            src = ps[:, 0:D].rearrange("p (h e) -> p h e", e=HD)
            dst = vaug_t[:, t, :].rearrange("p (h e) -> p h e",
                                            e=AUG)[:, :, 0:HD]
            nc.vector.tensor_copy(dst, src)

        # ---------------- attention units -------------------------------
        units = [(0, g) for g in range(NDT)] + [(1, g) for g in range(NDT)]
        kTs, vas = (kT1, kT2), (vaug1, vaug2)
        pt_tiles = {}   # (u, kjp) -> tile [P, 2, 2, N]

        def part1(u, kjp):
            """S + exp for kj pair kjp of unit u -> pt tile (bf16)."""
            br, g = units[u]
            kT_t = kTs[br]
            pt = pool_pt.tile([P, 2, 2, N], mdt, tag="pt")
            pt_tiles[(u, kjp)] = pt
            for kjl in range(2):
                kj = 2 * kjp + kjl
                for c in range(2):
                    sc = pp.tile([P, 2, 512], dt.float32, tag="S")
                    for hh in range(2):
                        r0 = hh * HD
                        nc.tensor.matmul(
                            sc[:, hh, :],
                            kT_t[r0:r0 + HD, g, kj * P:(kj + 1) * P],
                            qT[r0:r0 + HD, g, c * 512:(c + 1) * 512],
                            start=True, stop=True, skip_group_check=True)
                    nc.scalar.activation(
                        pt[:, :, kjl, c * 512:(c + 1) * 512], sc[:],
                        mybir.ActivationFunctionType.Exp, scale=SCALE)

        def emit_av(u, po, kjp):
            br, g = units[u]
            va = vas[br]
            pt = pt_tiles[(u, kjp)]
            for kjl in range(2):
                kj = 2 * kjp + kjl
                for hh in range(2):
                    h = 2 * g + hh
                    for c in range(2):
                        nc.tensor.matmul(
                            po[hh][0:AUG, c * 512:(c + 1) * 512],
                            va[:, kj, h * AUG:(h + 1) * AUG],
                            pt[:, hh, kjl, c * 512:(c + 1) * 512],
                            start=(kj == 0), stop=(kj == NQT - 1),
                            skip_group_check=True)

        def unit_copies(u, po):
            """evict AV result (unnormalized) + its row-sums; frees po.
            The last unit evicts on ACT (idle there) to shorten the tail."""
            br, g = units[u]
            bi, j = G2B[br][g]
            cp = nc.scalar.copy if u == NU - 1 else nc.vector.tensor_copy
            for hh in range(2):
                cp(ot[br][hh * HD:(hh + 1) * HD, g, :], po[hh][0:HD, :])
                rt = pool_sm.tile([1, N], mdt, tag="rt")
                cp(rt[:], po[hh][HD:HD + 1, :])
                rw = dram_rb.tile([8, P], mdt, tag="rw")
                nc.sync.dma_start(out=rw[:], in_=rt[:])
                row = 16 * j + 8 * hh
                nc.sync.dma_start(out=rall[(br, bi)][row:row + 8, :],
                                  in_=rw[:])

        def norm_batch(br, bi):
            """batched 1/r (partition-parallel) + broadcast + in-place scale."""
            gs = BATCHES[br][bi]
            rinv = pool_rv.tile([16 * len(gs), P], dt.float32, tag="rinv",
                                padded_shape=[48, P])
            nc.vector.reciprocal(rinv[:], rall[(br, bi)][:])
            rd = dram_rb.tile([2 * len(gs), N], dt.float32, tag="rd",
                              padded_shape=[6, N])
            nc.sync.dma_start(out=rd[:], in_=rinv[:])
            for jj, g in enumerate(gs):
                rb = pool_sm.tile([P, N], dt.float32, tag="rb")
                # 32-partition slices: partition_broadcast is DMA-descriptor
                # bound (~1/partition), so split across 4 parallel queues
                for q in range(4):
                    nc.sync.dma_start(
                        out=rb[q * 32:(q + 1) * 32, :],
                        in_=rd[2 * jj + q // 2, :].partition_broadcast(32))
                sl = ot[br][:, g, :]
                nc.vector.tensor_tensor(sl, sl, rb[:],
                                        mybir.AluOpType.mult)

        def proj_qi(br, qi, pool=None):
            if pool is None:
                psf = pp_o.tile([P, N], dt.float32, tag="O")
                ps = psf[:, 0:D]
            else:
                # tail projs run from the (idle) S pool so they don't wait
                # on the last unit's AV accumulators being evicted
                psf = pool.tile([P, 2, 512], dt.float32, tag="S")
                ps = psf.rearrange("p a n -> p (a n)")[:, 0:D]
            for g in range(NDT):
                osl = ot[br][:, g, qi * P:(qi + 1) * P]
                for c0, cn in ((0, 512), (512, 256)):
                    nc.tensor.matmul(
                        ps[:, c0:c0 + cn], osl, wp_t[:, g, c0:c0 + cn],
                        start=(g == 0), stop=(g == NDT - 1),
                        skip_group_check=True)
            res = pool_res.tile([P, D], dt.float32, tag="res")
            nc.vector.tensor_add(res[:], ps[:], bias_t[:])
            nc.sync.dma_start(out=out_d[br, qi * P:(qi + 1) * P, :],
                              in_=res[:])

        # ---------------- emission schedule -----------------------------
        # QKV-x: q^T, k^T
        for o in range(NDT):
            qkT_group(xt_t, 0, qT, o)
        for o in range(NDT):
            qkT_group(xt_t, D, kT1, o)
        # v interleaved with S/exp of unit 0 (needs only qT/kT1)
        for t in range(NQT):
            v_group(xt_t, vaug1, t)
            if t % 2 == 1:
                part1(0, t // 2)

        # mid-attention fillers: x2 QKV during branch-1, proj(br0) during
        # branch-2.  Safe points: QKV fillers mid-unit (depend only on the
        # past); proj fillers only at unit end (they wait on normalize).
        qkv_fill = ([(lambda o=o: qkT_group(x2t_t, D, kT2, o))
                     for o in range(NDT)]
                    + [(lambda t=t: v_group(x2t_t, vaug2, t))
                       for t in range(NQT)])
        proj_fill = []

        for u in range(NU):
            po = [pp_o.tile([P, N], dt.float32, tag="O",
                            name=f"po{u}_{hh}") for hh in range(2)]
            for kjp in range(4):
                emit_av(u, po, kjp)
                if u + 1 < NU:
                    part1(u + 1, kjp)
                if kjp and qkv_fill:
                    qkv_fill.pop(0)()
            unit_copies(u, po)
            br, g = units[u]
            for bi, gs in enumerate(BATCHES[br]):
                if g == gs[-1]:
                    norm_batch(br, bi)
            if u == 5:
                proj_fill = [(lambda qi=qi: proj_qi(0, qi))
                             for qi in range(NQT)]
            for _ in range({7: 1, 8: 1, 9: 1, 10: 1}.get(u, 0)):
                if proj_fill:
                    proj_fill.pop(0)()
        # tail: leftover br0 projs fill the last normalize window (from the
        # S psum pool, so they don't wait on the last AV eviction); then the
        # first two br1 projs run as g0-g4 partials during the g5 normalize
        # chain, finished with the g5 contribution once its scale lands.
        for qi in range(NQT - len(proj_fill), NQT):
            proj_qi(0, qi, pool=pp)

        def proj_partial(qi):
            psf = pp.tile([P, 2, 512], dt.float32, tag="S")
            ps = psf.rearrange("p a n -> p (a n)")[:, 0:D]
            for g in range(NDT - 1):
                osl = ot[1][:, g, qi * P:(qi + 1) * P]
                for c0, cn in ((0, 512), (512, 256)):
                    nc.tensor.matmul(
                        ps[:, c0:c0 + cn], osl, wp_t[:, g, c0:c0 + cn],
                        start=(g == 0), stop=False, skip_group_check=True)
            return ps

        def proj_finish(qi, ps):
            g = NDT - 1
            osl = ot[1][:, g, qi * P:(qi + 1) * P]
            for c0, cn in ((0, 512), (512, 256)):
                nc.tensor.matmul(
                    ps[:, c0:c0 + cn], osl, wp_t[:, g, c0:c0 + cn],
                    start=False, stop=True, skip_group_check=True)
            res = pool_res.tile([P, D], dt.float32, tag="res")
            nc.vector.tensor_add(res[:], ps[:], bias_t[:])
            nc.sync.dma_start(out=out_d[1, qi * P:(qi + 1) * P, :],
                              in_=res[:])

        parts = [proj_partial(qi) for qi in (0, 1)]
        for qi in (0, 1):
            proj_finish(qi, parts[qi])
        for qi in range(2, NQT):
            proj_qi(1, qi)

    n = _split_multiwaits(nc)
    _built = (nc, n)
    return _built


def _host_prep(x, x2, qkv_w, proj_w, proj_b):
    """-> list of 8 per-core input maps (bf16 operands, f32 bias)."""
    import ml_dtypes
    bf = lambda a: np.ascontiguousarray(np.asarray(a),
                                        ).astype(ml_dtypes.bfloat16)

    xt = np.transpose(np.asarray(x), (0, 2, 1))
    x2t = np.transpose(np.asarray(x2), (0, 2, 1))
    wqk = bf(np.asarray(qkv_w)[:2 * D].T)       # [768, 1536]
    wv = bf(np.asarray(qkv_w)[2 * D:].T)        # [768, 768]
    wp = bf(np.asarray(proj_w).T)               # [768, 768]
    bias = np.broadcast_to(np.asarray(proj_b, dtype=np.float32),
                           (P, D)).copy()
    ones = np.ones((P, H, 1), dtype=ml_dtypes.bfloat16)
    maps = []
    for c in range(B):
        maps.append({
            "xt": bf(xt[c]), "x2t": bf(x2t[c]),
            "wqk": wqk, "wv": wv, "wp": wp, "bias": bias,
            "ones": ones,
        })
    return maps


def kernel(x, x2, qkv_w, proj_w, proj_b, trace=False, tmpdir=None):
    nc, _ = _build()
    from concourse.bass_utils import run_bass_kernel_spmd
    in_maps = _host_prep(x, x2, qkv_w, proj_w, proj_b)
    res = run_bass_kernel_spmd(nc, in_maps, list(range(B)), trace=trace,
                               tmpdir=tmpdir)
    kernel.last_exec_time_ns = res.exec_time_ns
    out = np.stack([res.results[c]["out"] for c in range(B)])  # [B,2,N,D]
    out1 = np.ascontiguousarray(out[:, 0])
    out2 = np.ascontiguousarray(out[:, 1])
    return (out1, out2)


kernel.last_exec_time_ns = None


# revision 48
# speedup vs baseline: 1.0095x; 1.0063x over previous
"""Two-branch attention kernel for Trainium2 (8 NeuronCores, batch-parallel).

out1 = proj(softmax(q k^T / 8) v),  out2 = proj(softmax(q k2^T / 8) v2)
with q,k,v from x and k2,v2 from x2 (q shared across branches).

Sharding: batch dim (8) -> one batch element per core. No collectives.

Design (vs f32r baseline at 544us; this kernel ~355us at full clock):
  * all matmul operands bf16: halves DMA + SBUF, no DRAM spill of k2/v2
    (everything resident), faster LDWEIGHTS. rel err ~6e-3 (budget 2e-2).
  * S psum split into [P,2,512] half-tiles double-buffered so exp (ACT)
    overlaps the next S matmul instead of serializing the PE; keeping the
    PE gap-free also keeps it at the 2.4GHz p-state (vs 1.2GHz).
  * softmax normalization decoupled from the AV accumulators: unnormalized
    o + row-sum rows are evicted immediately (frees PSUM), row sums are
    gathered via DRAM-bounce DMAs into [8,128]-reshaped batch tiles so one
    batched DVE reciprocal covers 6 heads in ~0.9us (reciprocal costs
    free-size cycles: 6.5us for [n,1024], regardless of n), then 1/r is
    partition-broadcast and applied in-place on DVE.
  * emission-order software pipelining: QKV for x2 (k2T/v2) interleaved
    into branch-1 attention; branch-1 proj into branch-2 attention; S/exp
    for the next (branch,g) unit interleaved into the current unit's AV;
    all PSUM evictions on DVE so ACT does nothing but exp in steady state;
    4 branch-0 projs reserved for the tail (from the idle S psum pool) to
    cover the last normalize chain.
"""
import sys
for _p in ('/opt/trn_rl_repo',):
    if _p not in sys.path:
        sys.path.insert(0, _p)

import numpy as np

MODE = 'bf16-pipelined'

B, N, D, H, HD = 8, 1024, 768, 12, 64
SCALE = HD ** -0.5
NDT = D // 128       # 6 dim tiles
NQT = N // 128       # 8 token tiles
P = 128
AUG = HD + 1         # 65: head dim + ones column for row sums
NU = 12              # (branch, g) attention units


# ----------------------------------------------------------------------------
# workaround: walrus rejects >2 sem waits on one instruction; TileContext's
# tail drain carries one wait per active logical proc. Split them across
# single-wait SP nops and emit a bare drain.
def _install_tilefix():
    import bass_rust
    import concourse.tile as tile

    def _drain_and_barrier_split(self, tick_clock, wait_clock):
        gc = tick_clock.global_clock
        ticks = [gc[i] for i in range(27)]
        for i, t in enumerate(ticks):
            if t > 0:
                vc = bass_rust.VectorClock(
                    [t if j == i else 0 for j in range(len(ticks))])
                nop = self.nc.sync.nop()
                wait_clock.add_sem_waits(
                    nop.ins, bass_rust.ScopedClock({None: vc}))
        self.nc.sync.drain()
        self.nc.all_engine_barrier()
        assert self.sems is not None
        popped = self.nc._tile_sem_poison_stack.pop()
        assert popped is self._sem_poison
        self.nc.clear_and_free_semaphores(list(self.sems.allocated().values()))
        self.nc.all_engine_barrier()

    tile.TileContext._drain_and_barrier = _drain_and_barrier_split


def _split_multiwaits(nc, max_waits=1):
    """walrus codegen rejects instructions carrying more than `max_waits`
    sync waits; hoist the extras onto same-engine nops placed just before."""
    import bass_rust
    import concourse.mybir as mybir
    cnt = 0
    for bb in nc.main_func.blocks:
        insts = bb.instructions
        i = 0
        while i < len(insts):
            ins = insts[i]
            si = getattr(ins, 'sync_info', None)
            if si is not None and si.on_wait and len(si.on_wait) > max_waits:
                waits = list(si.on_wait)
                extras, keep = waits[:-max_waits], waits[-max_waits:]
                for w in extras:
                    nop = mybir.InstNoOp(name=f"I-swx{cnt}", ins=[], outs=[])
                    cnt += 1
                    nop.engine = ins.engine
                    nop.sync_info = bass_rust.SyncInfo(on_wait=[w],
                                                       on_update=[])
                    insts.insert(i, nop)
                    i += 1
                ins.sync_info = bass_rust.SyncInfo(
                    on_wait=keep, on_update=list(si.on_update))
            i += 1
    return cnt


_built = None


def _build():
    """Build the SPMD bass program once. Returns (nc, n_split_waits)."""
    global _built
    if _built is not None:
        return _built
    _install_tilefix()
    from contextlib import ExitStack
    import concourse.bass as bass
    import concourse.tile as tile
    from concourse import mybir

    dt = mybir.dt
    mdt = dt.bfloat16          # matmul operand dtype everywhere

    nc = bass.Bass("TRN2", target_bir_lowering=False, debug=False,
                   num_devices=8)

    # DRAM I/O (per core)
    xt_d = nc.dram_tensor("xt", [D, N], mdt, kind="ExternalInput")
    x2t_d = nc.dram_tensor("x2t", [D, N], mdt, kind="ExternalInput")
    wqk_d = nc.dram_tensor("wqk", [D, 2 * D], mdt, kind="ExternalInput")
    wv_d = nc.dram_tensor("wv", [D, D], mdt, kind="ExternalInput")
    wp_d = nc.dram_tensor("wp", [D, D], mdt, kind="ExternalInput")
    bias_d = nc.dram_tensor("bias", [P, D], dt.float32, kind="ExternalInput")
    ones_d = nc.dram_tensor("ones", [P, H, 1], mdt, kind="ExternalInput")
    out_d = nc.dram_tensor("out", [2, N, D], dt.float32,
                           kind="ExternalOutput")

    with tile.TileContext(nc) as tc, ExitStack() as top:
        # PSUM: pp (S half-tiles + QKV groups, 2KB ea) 4 banks,
        #       pp_o (AV accumulators + proj) 4 banks.
        pp = top.enter_context(tc.tile_pool(name="ps", bufs=2, space="PSUM"))
        pp_o = top.enter_context(tc.tile_pool(name="ps_o", bufs=2,
                                              space="PSUM"))
        dram_rb = top.enter_context(tc.tile_pool(name="dram_rb", bufs=2,
                                                 space="DRAM"))
        persist = top.enter_context(tc.tile_pool(name="persist", bufs=1))
        pool_pt = top.enter_context(tc.tile_pool(name="pt", bufs=5))
        pool_sm = top.enter_context(tc.tile_pool(name="sm", bufs=2))
        pool_rv = top.enter_context(tc.tile_pool(name="rv", bufs=1))
        pool_res = top.enter_context(tc.tile_pool(name="res", bufs=2))

        # persistent SBUF tiles (bf16): ~104KB/partition
        qT = persist.tile([P, NDT, N], mdt, tag="qT")
        kT1 = persist.tile([P, NDT, N], mdt, tag="kT1")
        kT2 = persist.tile([P, NDT, N], mdt, tag="kT2")
        vaug1 = persist.tile([P, NQT, H * AUG], mdt, tag="va1")
        vaug2 = persist.tile([P, NQT, H * AUG], mdt, tag="va2")
        wp_t = persist.tile([P, NDT, D], mdt, tag="wp")
        bias_t = persist.tile([P, D], dt.float32, tag="bias")
        ot = [persist.tile([P, NDT, N], mdt, tag=f"ot{b}", name=f"ot{b}")
              for b in (0, 1)]
        # r rows, gathered via SBUF-to-SBUF DMA (engines cannot write at
        # arbitrary partition bases; DMA can) and reshaped [row,1024] ->
        # [8 partitions,128] so the slow reciprocal runs partition-parallel.
        # br0 batches complete at units 2/5; br1 at 8/10/11 (small last
        # batch keeps the tail chain short).
        BATCHES = {0: [(0, 1, 2), (3, 4, 5)],
                   1: [(0, 1, 2), (3,), (4,), (5,)]}
        G2B = {br: {g: (bi, list(gs).index(g))
                    for bi, gs in enumerate(BATCHES[br]) for g in gs}
               for br in (0, 1)}
        # [8 partitions, 128] blocks per r-vector: the slow DVE reciprocal
        # costs free-size cycles, so folding tokens onto partitions makes
        # the batched reciprocal ~8x cheaper. Gather goes through DRAM
        # (SBUF->SBUF partition-reshape DMAs fail to load).
        rall = {(br, bi): persist.tile([16 * len(gs), P], mdt,
                                       tag=f"rall{br}{bi}",
                                       name=f"rall{br}{bi}")
                for br in (0, 1) for bi, gs in enumerate(BATCHES[br])}

        # phase-A inputs (innermost pool; closed once QKV emission is done)
        pha = top.enter_context(tc.tile_pool(name="pha", bufs=1))
        xt_t = pha.tile([P, NDT, N], mdt, tag="xt")
        x2t_t = pha.tile([P, NDT, N], mdt, tag="x2t")
        wqk_t = pha.tile([P, NDT, 2 * D], mdt, tag="wqk")
        wv_t = pha.tile([P, NDT, D], mdt, tag="wv")

        # input DMAs, priority order; q-columns chunked per output tile so
        # the first matmul group starts after ~1/12 of the weights arrive
        nc.sync.dma_start(
            out=wqk_t[:, :, 0:P],
            in_=wqk_d[:, 0:P].rearrange("(i p) d -> p i d", p=P))
        for i in range(NDT):
            nc.sync.dma_start(out=xt_t[:, i, :],
                              in_=xt_d[i * P:(i + 1) * P, :])
        for o in range(1, NDT):
            nc.sync.dma_start(
                out=wqk_t[:, :, o * P:(o + 1) * P],
                in_=wqk_d[:, o * P:(o + 1) * P].rearrange(
                    "(i p) d -> p i d", p=P))
        nc.sync.dma_start(
            out=wqk_t[:, :, D:2 * D],
            in_=wqk_d[:, D:2 * D].rearrange("(i p) d -> p i d", p=P))
        nc.sync.dma_start(out=wv_t,
                          in_=wv_d[:].rearrange("(i p) d -> p i d", p=P))
        nc.sync.dma_start(out=x2t_t,
                          in_=x2t_d[:].rearrange("(i p) n -> p i n", p=P))
        nc.sync.dma_start(
            out=wp_t, in_=wp_d[:].rearrange("(g p) d -> p g d", p=P))
        nc.sync.dma_start(out=bias_t, in_=bias_d[:])
        for va in (vaug1, vaug2):
            for t in range(NQT):
                nc.sync.dma_start(
                    out=va[:, t, :].rearrange("p (h e) -> p h e",
                                              e=AUG)[:, :, HD:AUG],
                    in_=ones_d[:])

        # ---------------- QKV emit units --------------------------------
        def qkT_group(src_x, wcol0, dst, o):
            """one [128,1024] output tile of q^T/k^T via W-stationary."""
            psf = pp.tile([P, 2, 512], dt.float32, tag="S")
            ps = psf.rearrange("p a n -> p (a n)")
            for i in range(NDT):
                wsl = wqk_t[:, i, wcol0 + o * P: wcol0 + (o + 1) * P]
                for c in range(2):
                    nc.tensor.matmul(
                        ps[:, c * 512:(c + 1) * 512], wsl,
                        src_x[:, i, c * 512:(c + 1) * 512],
                        start=(i == 0), stop=(i == NDT - 1))
            nc.vector.tensor_copy(dst[:, o, :], ps[:])

        def v_group(src_x, vaug_t, t):
            """one [128tok, 768] v tile via x-stationary into vaug."""
            psf = pp.tile([P, 2, 512], dt.float32, tag="S")
            ps = psf.rearrange("p a n -> p (a n)")
            for i in range(NDT):
                xsl = src_x[:, i, t * P:(t + 1) * P]
                for c0, cn in ((0, 512), (512, 256)):
                    nc.tensor.matmul(
                        ps[:, c0:c0 + cn], xsl, wv_t[:, i, c0:c0 + cn],
                        start=(i == 0), stop=(i == NDT - 1))
            src = ps[:, 0:D].rearrange("p (h e) -> p h e", e=HD)
            dst = vaug_t[:, t, :].rearrange("p (h e) -> p h e",
                                            e=AUG)[:, :, 0:HD]
            nc.vector.tensor_copy(dst, src)

        # ---------------- attention units -------------------------------
        units = [(0, g) for g in range(NDT)] + [(1, g) for g in range(NDT)]
        kTs, vas = (kT1, kT2), (vaug1, vaug2)
        pt_tiles = {}   # (u, kjp) -> tile [P, 2, 2, N]

        def part1(u, kjp):
            """S + exp for kj pair kjp of unit u -> pt tile (bf16)."""
            br, g = units[u]
            kT_t = kTs[br]
            pt = pool_pt.tile([P, 2, 2, N], mdt, tag="pt")
            pt_tiles[(u, kjp)] = pt
            for kjl in range(2):
                kj = 2 * kjp + kjl
                for c in range(2):
                    sc = pp.tile([P, 2, 512], dt.float32, tag="S")
                    for hh in range(2):
                        r0 = hh * HD
                        nc.tensor.matmul(
                            sc[:, hh, :],
                            kT_t[r0:r0 + HD, g, kj * P:(kj + 1) * P],
                            qT[r0:r0 + HD, g, c * 512:(c + 1) * 512],
                            start=True, stop=True, skip_group_check=True)
                    nc.scalar.activation(
                        pt[:, :, kjl, c * 512:(c + 1) * 512], sc[:],
                        mybir.ActivationFunctionType.Exp, scale=SCALE)

        def emit_av(u, po, kjp):
            br, g = units[u]
            va = vas[br]
            pt = pt_tiles[(u, kjp)]
            for kjl in range(2):
                kj = 2 * kjp + kjl
                for hh in range(2):
                    h = 2 * g + hh
                    for c in range(2):
                        nc.tensor.matmul(
                            po[hh][0:AUG, c * 512:(c + 1) * 512],
                            va[:, kj, h * AUG:(h + 1) * AUG],
                            pt[:, hh, kjl, c * 512:(c + 1) * 512],
                            start=(kj == 0), stop=(kj == NQT - 1),
                            skip_group_check=True)

        def unit_copies(u, po):
            """evict AV result (unnormalized) + its row-sums; frees po.
            The last unit evicts on ACT (idle there) to shorten the tail."""
            br, g = units[u]
            bi, j = G2B[br][g]
            cp = nc.scalar.copy if u == NU - 1 else nc.vector.tensor_copy
            for hh in range(2):
                cp(ot[br][hh * HD:(hh + 1) * HD, g, :], po[hh][0:HD, :])
                rt = pool_sm.tile([1, N], mdt, tag="rt")
                cp(rt[:], po[hh][HD:HD + 1, :])
                rw = dram_rb.tile([8, P], mdt, tag="rw")
                nc.sync.dma_start(out=rw[:], in_=rt[:])
                row = 16 * j + 8 * hh
                nc.sync.dma_start(out=rall[(br, bi)][row:row + 8, :],
                                  in_=rw[:])

        def norm_batch(br, bi):
            """batched 1/r (partition-parallel) + broadcast + in-place scale."""
            gs = BATCHES[br][bi]
            rinv = pool_rv.tile([16 * len(gs), P], dt.float32, tag="rinv",
                                padded_shape=[48, P])
            nc.vector.reciprocal(rinv[:], rall[(br, bi)][:])
            rd = dram_rb.tile([2 * len(gs), N], dt.float32, tag="rd",
                              padded_shape=[6, N])
            nc.sync.dma_start(out=rd[:], in_=rinv[:])
            for jj, g in enumerate(gs):
                rb = pool_sm.tile([P, N], dt.float32, tag="rb")
                # 32-partition slices: partition_broadcast is DMA-descriptor
                # bound (~1/partition), so split across 4 parallel queues
                for q in range(4):
                    nc.sync.dma_start(
                        out=rb[q * 32:(q + 1) * 32, :],
                        in_=rd[2 * jj + q // 2, :].partition_broadcast(32))
                sl = ot[br][:, g, :]
                nc.vector.tensor_tensor(sl, sl, rb[:],
                                        mybir.AluOpType.mult)

        def proj_qi(br, qi, pool=None):
            if pool is None:
                psf = pp_o.tile([P, N], dt.float32, tag="O")
                ps = psf[:, 0:D]
            else:
                # tail projs run from the (idle) S pool so they don't wait
                # on the last unit's AV accumulators being evicted
                psf = pool.tile([P, 2, 512], dt.float32, tag="S")
                ps = psf.rearrange("p a n -> p (a n)")[:, 0:D]
            for g in range(NDT):
                osl = ot[br][:, g, qi * P:(qi + 1) * P]
                for c0, cn in ((0, 512), (512, 256)):
                    nc.tensor.matmul(
                        ps[:, c0:c0 + cn], osl, wp_t[:, g, c0:c0 + cn],
                        start=(g == 0), stop=(g == NDT - 1),
                        skip_group_check=True)
            res = pool_res.tile([P, D], dt.float32, tag="res")
            nc.vector.tensor_add(res[:], ps[:], bias_t[:])
            nc.sync.dma_start(out=out_d[br, qi * P:(qi + 1) * P, :],
                              in_=res[:])

        # ---------------- emission schedule -----------------------------
        # QKV-x: q^T, k^T
        for o in range(NDT):
            qkT_group(xt_t, 0, qT, o)
        for o in range(NDT):
            qkT_group(xt_t, D, kT1, o)
        # v interleaved with S/exp of unit 0 (needs only qT/kT1)
        for t in range(NQT):
            v_group(xt_t, vaug1, t)
            if t % 2 == 1:
                part1(0, t // 2)

        # mid-attention fillers: x2 QKV during branch-1, proj(br0) during
        # branch-2.  Safe points: QKV fillers mid-unit (depend only on the
        # past); proj fillers only at unit end (they wait on normalize).
        qkv_fill = ([(lambda o=o: qkT_group(x2t_t, D, kT2, o))
                     for o in range(NDT)]
                    + [(lambda t=t: v_group(x2t_t, vaug2, t))
                       for t in range(NQT)])
        proj_fill = []

        for u in range(NU):
            po = [pp_o.tile([P, N], dt.float32, tag="O",
                            name=f"po{u}_{hh}") for hh in range(2)]
            for kjp in range(4):
                # S/exp of the next unit first: its exps are the pace-setter
                # (ACT-bound region), so don't park them behind 8 AV matmuls
                if u + 1 < NU:
                    part1(u + 1, kjp)
                emit_av(u, po, kjp)
                if kjp and qkv_fill:
                    qkv_fill.pop(0)()
            unit_copies(u, po)
            br, g = units[u]
            for bi, gs in enumerate(BATCHES[br]):
                if g == gs[-1]:
                    norm_batch(br, bi)
            if u == 5:
                proj_fill = [(lambda qi=qi: proj_qi(0, qi))
                             for qi in range(NQT)]
            for _ in range({7: 1, 8: 1, 9: 1, 10: 1}.get(u, 0)):
                if proj_fill:
                    proj_fill.pop(0)()
        # tail: leftover br0 projs fill the last normalize window (from the
        # S psum pool, so they don't wait on the last AV eviction); then the
        # first two br1 projs run as g0-g4 partials during the g5 normalize
        # chain, finished with the g5 contribution once its scale lands.
        for qi in range(NQT - len(proj_fill), NQT):
            proj_qi(0, qi, pool=pp)

        def proj_partial(qi):
            psf = pp.tile([P, 2, 512], dt.float32, tag="S")
            ps = psf.rearrange("p a n -> p (a n)")[:, 0:D]
            for g in range(NDT - 1):
                osl = ot[1][:, g, qi * P:(qi + 1) * P]
                for c0, cn in ((0, 512), (512, 256)):
                    nc.tensor.matmul(
                        ps[:, c0:c0 + cn], osl, wp_t[:, g, c0:c0 + cn],
                        start=(g == 0), stop=False, skip_group_check=True)
            return ps

        def proj_finish(qi, ps):
            g = NDT - 1
            osl = ot[1][:, g, qi * P:(qi + 1) * P]
            for c0, cn in ((0, 512), (512, 256)):
                nc.tensor.matmul(
                    ps[:, c0:c0 + cn], osl, wp_t[:, g, c0:c0 + cn],
                    start=False, stop=True, skip_group_check=True)
            res = pool_res.tile([P, D], dt.float32, tag="res")
            nc.vector.tensor_add(res[:], ps[:], bias_t[:])
            nc.sync.dma_start(out=out_d[1, qi * P:(qi + 1) * P, :],
                              in_=res[:])

        parts = [proj_partial(qi) for qi in (0, 1)]
        for qi in (0, 1):
            proj_finish(qi, parts[qi])
        for qi in range(2, NQT):
            proj_qi(1, qi)

    n = _split_multiwaits(nc)
    _built = (nc, n)
    return _built


def _host_prep(x, x2, qkv_w, proj_w, proj_b):
    """-> list of 8 per-core input maps (bf16 operands, f32 bias)."""
    import ml_dtypes
    bf = lambda a: np.ascontiguousarray(np.asarray(a),
                                        ).astype(ml_dtypes.bfloat16)

    xt = np.transpose(np.asarray(x), (0, 2, 1))
    x2t = np.transpose(np.asarray(x2), (0, 2, 1))
    wqk = bf(np.asarray(qkv_w)[:2 * D].T)       # [768, 1536]
    wv = bf(np.asarray(qkv_w)[2 * D:].T)        # [768, 768]
    wp = bf(np.asarray(proj_w).T)               # [768, 768]
    bias = np.broadcast_to(np.asarray(proj_b, dtype=np.float32),
                           (P, D)).copy()
    ones = np.ones((P, H, 1), dtype=ml_dtypes.bfloat16)
    maps = []
    for c in range(B):
        maps.append({
            "xt": bf(xt[c]), "x2t": bf(x2t[c]),
            "wqk": wqk, "wv": wv, "wp": wp, "bias": bias,
            "ones": ones,
        })
    return maps


def kernel(x, x2, qkv_w, proj_w, proj_b, trace=False, tmpdir=None):
    nc, _ = _build()
    from concourse.bass_utils import run_bass_kernel_spmd
    in_maps = _host_prep(x, x2, qkv_w, proj_w, proj_b)
    res = run_bass_kernel_spmd(nc, in_maps, list(range(B)), trace=trace,
                               tmpdir=tmpdir)
    kernel.last_exec_time_ns = res.exec_time_ns
    out = np.stack([res.results[c]["out"] for c in range(B)])  # [B,2,N,D]
    out1 = np.ascontiguousarray(out[:, 0])
    out2 = np.ascontiguousarray(out[:, 1])
    return (out1, out2)


kernel.last_exec_time_ns = None
